# revision 1
# baseline (speedup 1.0000x reference)
"""AdaptiveGraphPooling Trainium2 kernel (8 NeuronCores, SPMD).

Strategy
--------
Graphs are sharded 256-per-core (nodes sorted by graph id, so each core gets a
contiguous node range of whole graphs).  Each graph is padded on the host to a
fixed LFIX=576-node slot (zeros), making every per-graph slice *static* and
identical across the 8 cores -> one SPMD instruction stream.  Per-slot work is
trimmed to lmax[g] = max over cores of that slot's real length (baked into the
instruction stream at kernel build; identical on all cores).

Per core (bf16 data paths, fp32 accumulation):
  - stream xT [128, NPAD] feature-major, in PAIRS of 16-graph groups
  - PE: h^T = att_W.T @ xT, pair-packed onto 128 psum partitions (two
    matmuls with zero-padded lhsT halves, accumulating)
  - ACT: th = tanh(h^T + att_b) on full [128, 512] tiles
  - PE per group: scores packed as psum ROWS (row j = graph j) via one-hot
    ctx-column lhsT tiles (zero-columns accumulate harmlessly)
  - ACT per group: E-rows = Exp(scores), accum_out -> z16 (softmax
    denominators, free); DMA-reshape E-rows -> node-major e_row [1, CH]
  - PE per graph: broadcast e across partitions with a K=1 matmul
    (ones[1,128] (x) e_row-slot) into PSUM
  - DVE/ACT per graph (slices trimmed to lmax[g]):
      max  = tensor_scalar accum max          (DVE)
      sum  = tensor_scalar accum add (DVE) or Copy accum (ACT, load balance)
      exsum= scalar_tensor_tensor x*E accum add (DVE, in1 reads PSUM)
  - padding nodes: x=0 -> sum/exsum +0; max vs 0 (true max > 0 whp);
    z corrected by host constant (LFIX-counts)*e_pad
  - final: per-128-graph-block normalize, transpose, 3x128 selector MLP,
    softmax over 3 strategies, mix -> out [256, 128] f32
Host concatenates per-core outputs -> [2048, 128] f32. No collectives.
"""

import math
import numpy as np
import ml_dtypes

from concourse import bass, bacc, mybir
from concourse import tile as tile_mod
from concourse.bass_utils import run_bass_kernel_spmd

BF16 = mybir.dt.bfloat16
F32 = mybir.dt.float32
ALU = mybir.AluOpType
ACTF = mybir.ActivationFunctionType

D = 128
A = 64
NCORES = 8
LFIX = 576
GROUP = 16

# fraction of per-graph sum ops issued on ACT instead of DVE (load balance)
SUM_ACT_FRAC = 0.70


def build_nc(LFIX_, GPC, GROUP_, lmax):
    """Build the SPMD bass program.

    lmax: tuple of GPC ints - per-slot trimmed length (multiple of 4,
          512 < lmax[g] <= LFIX). Identical across cores by construction.
    """
    LF = LFIX_
    G = GROUP_
    assert GPC % 128 == 0 and GPC % (2 * G) == 0
    NPAD = GPC * LF
    NGRP = GPC // G
    CH = G * LF                     # nodes per group
    NBLK = GPC // 128
    assert CH % 1024 == 0
    lmax = tuple(int(v) for v in lmax)
    assert len(lmax) == GPC and all(512 < v <= LF for v in lmax)

    nc = bacc.Bacc(None, target_bir_lowering=False, debug=False)

    xT = nc.declare_dram_parameter("xT", [D, NPAD], BF16, isOutput=False)
    # attW2: [128, 128]; cols 0:64 = att_W (-> psum partitions 0:64),
    # cols 64:128 = att_W (-> partitions 64:128).  Used with accumulate so
    # each matmul of a pair fills one partition half.
    attWlo = nc.declare_dram_parameter("attWlo", [D, D], BF16, isOutput=False)
    attWhi = nc.declare_dram_parameter("attWhi", [D, D], BF16, isOutput=False)
    attb = nc.declare_dram_parameter("attb", [D, 1], F32, isOutput=False)
    # ctxoh lo/hi: per graph-in-group j a [128, G] one-hot-column tile with
    # att_ctx in column j, placed at rows 0:64 (lo) or rows 64:128 (hi).
    ctxlo = nc.declare_dram_parameter("ctxlo", [D, G * G], BF16,
                                      isOutput=False)
    ctxhi = nc.declare_dram_parameter("ctxhi", [D, G * G], BF16,
                                      isOutput=False)
    outW = nc.declare_dram_parameter("outW", [D, D], BF16, isOutput=False)
    outb = nc.declare_dram_parameter("outb", [D, 1], F32, isOutput=False)
    selW1 = nc.declare_dram_parameter("selW1", [3 * D, D], BF16, isOutput=False)
    selb1 = nc.declare_dram_parameter("selb1", [D, 1], F32, isOutput=False)
    selW2 = nc.declare_dram_parameter("selW2", [D, 3], BF16, isOutput=False)
    selb2 = nc.declare_dram_parameter("selb2", [3, 1], F32, isOutput=False)
    countsP = nc.declare_dram_parameter("counts", [GPC, 1], F32, isOutput=False)
    zcorrP = nc.declare_dram_parameter("zcorr", [GPC, 1], F32, isOutput=False)
    identP = nc.declare_dram_parameter("ident", [D, D], F32, isOutput=False)
    outP = nc.declare_dram_parameter("out", [GPC, D], F32, isOutput=True)

    with tile_mod.TileContext(nc) as tc:
        with tc.tile_pool(name="const", bufs=1) as cpool:
            attWlo_sb = cpool.tile([D, D], BF16)
            nc.sync.dma_start(attWlo_sb[:], attWlo[:])
            attWhi_sb = cpool.tile([D, D], BF16)
            nc.sync.dma_start(attWhi_sb[:], attWhi[:])
            attb_sb = cpool.tile([D, 1], F32)
            nc.sync.dma_start(attb_sb[:], attb[:])
            ctxlo_sb = cpool.tile([D, G * G], BF16)
            nc.sync.dma_start(ctxlo_sb[:], ctxlo[:])
            ctxhi_sb = cpool.tile([D, G * G], BF16)
            nc.sync.dma_start(ctxhi_sb[:], ctxhi[:])
            ones1_sb = cpool.tile([1, D], BF16)
            nc.vector.memset(ones1_sb[:], 1.0)
            outW_sb = cpool.tile([D, D], BF16)
            nc.sync.dma_start(outW_sb[:], outW[:])
            outb_sb = cpool.tile([D, 1], F32)
            nc.sync.dma_start(outb_sb[:], outb[:])
            selW1_sb = cpool.tile([D, 3 * D], BF16)
            for k in range(3):
                nc.sync.dma_start(
                    selW1_sb[:, k * D:(k + 1) * D], selW1[k * D:(k + 1) * D, :]
                )
            selb1_sb = cpool.tile([D, 1], F32)
            nc.sync.dma_start(selb1_sb[:], selb1[:])
            selW2_sb = cpool.tile([D, 3], BF16)
            nc.sync.dma_start(selW2_sb[:], selW2[:])
            selb2_sb = cpool.tile([3, 1], F32)
            nc.sync.dma_start(selb2_sb[:], selb2[:])
            ident_sb = cpool.tile([D, D], F32)
            nc.sync.dma_start(ident_sb[:], identP[:])

            counts_bl = []
            zcorr_bl = []
            for b in range(NBLK):
                cb = cpool.tile([128, 1], F32, name=f"counts_b{b}")
                nc.sync.dma_start(cb[:], countsP[b * 128:(b + 1) * 128, :])
                counts_bl.append(cb)
                zb = cpool.tile([128, 1], F32, name=f"zcorr_b{b}")
                nc.sync.dma_start(zb[:], zcorrP[b * 128:(b + 1) * 128, :])
                zcorr_bl.append(zb)

            sumT, maxT, exT, zcolB = [], [], [], []
            for b in range(NBLK):
                sumT.append(cpool.tile([128, 128], F32, name=f"sumT{b}"))
                maxT.append(cpool.tile([128, 128], F32, name=f"maxT{b}"))
                exT.append(cpool.tile([128, 128], F32, name=f"exT{b}"))
                zcolB.append(cpool.tile([128, 1], F32, name=f"zcol{b}"))

            # ---- main streaming loop: pairs of 16-graph groups -----------
            with (
                tc.tile_pool(name="xstream", bufs=2) as xpool,
                tc.tile_pool(name="thc", bufs=2) as thpool,
                tc.tile_pool(name="esb", bufs=3) as epool,
                tc.tile_pool(name="scr", bufs=8) as spool,
                tc.tile_pool(name="thp", bufs=2, space="PSUM") as thps,
                tc.tile_pool(name="sps", bufs=1, space="PSUM") as sps_pool,
                tc.tile_pool(name="eps", bufs=2, space="PSUM") as eps_pool,
            ):
                for pk in range(NGRP // 2):
                    xts = []
                    for h in (0, 1):
                        gi = 2 * pk + h
                        xt = xpool.tile([D, CH], BF16, tag=f"xt{h}",
                                        name=f"xt{gi}")
                        nc.sync.dma_start(
                            xt[:], xT[:, gi * CH:(gi + 1) * CH])
                        xts.append(xt)

                    # pair-packed h^T + tanh on full 128 partitions
                    thc = thpool.tile([D, CH], BF16, tag="thc", name=f"thc{pk}")
                    for t0 in range(0, CH, 512):
                        thp = thps.tile([D, 512], F32, tag="thp")
                        # pair-packed: lo fills partitions 0:64 (hi cols are
                        # zero), hi accumulates partitions 64:128
                        nc.tensor.matmul(
                            thp[:], attWlo_sb[:],
                            xts[0][:, t0:t0 + 512],
                            start=True, stop=False,
                        )
                        nc.tensor.matmul(
                            thp[:], attWhi_sb[:],
                            xts[1][:, t0:t0 + 512],
                            start=False, stop=True,
                        )
                        nc.scalar.activation(
                            thc[:, t0:t0 + 512], thp[:], ACTF.Tanh,
                            bias=attb_sb[:], scale=1.0,
                        )

                    for h in (0, 1):
                        gi = 2 * pk + h
                        xt = xts[h]
                        ctx_sb = ctxlo_sb if h == 0 else ctxhi_sb
                        # scores for the whole 16-graph group, packed as psum
                        # rows; full LF width so padding cols hold s_pad
                        sps = sps_pool.tile([G, LF], F32, tag="sps")
                        for j in range(G):
                            goff = j * LF
                            lhs = ctx_sb[:, j * G:(j + 1) * G]
                            nc.tensor.matmul(
                                sps[:, 0:512], lhs,
                                thc[:, goff:goff + 512],
                                start=(j == 0), stop=False,
                            )
                            nc.tensor.matmul(
                                sps[:, 512:LF], lhs,
                                thc[:, goff + 512:goff + LF],
                                start=(j == 0), stop=(j == G - 1),
                            )
                        # E rows = exp(s); accum -> z16; reshape to e_row
                        esb = epool.tile([G, LF], BF16, tag="esb")
                        z16 = epool.tile([G, 1], F32, tag="z16")
                        nc.scalar.activation(
                            esb[:], sps[:], ACTF.Exp, accum_out=z16[:],
                        )
                        b = (gi * G) // 128
                        brow = ((gi * G) % 128)
                        nc.sync.dma_start(
                            zcolB[b][brow:brow + G, 0:1], z16[:])
                        erow = epool.tile([1, CH], BF16, tag="erow")
                        nc.sync.dma_start(erow[0:1, :], esb[:])

                        for j in range(G):
                            g = gi * G + j
                            b = g // 128
                            col = g % 128
                            lm = lmax[g]
                            goff = j * LF
                            # broadcast e across partitions via K=1 matmul
                            eps = eps_pool.tile([D, LF], F32, tag="eps")
                            nc.tensor.matmul(
                                eps[:, 0:512], ones1_sb[:],
                                erow[0:1, goff:goff + 512],
                                start=True, stop=True,
                            )
                            nc.tensor.matmul(
                                eps[:, 512:lm], ones1_sb[:],
                                erow[0:1, goff + 512:goff + lm],
                                start=True, stop=True,
                            )

                            xslot = xt[:, goff:goff + lm]
                            scr = spool.tile([D, LF], BF16, tag="scr")
                            if (g % 20) < int(SUM_ACT_FRAC * 20):
                                nc.scalar.activation(
                                    scr[:, 0:lm], xslot, ACTF.Copy,
                                    accum_out=sumT[b][:, col:col + 1],
                                )
                            else:
                                nc.vector.tensor_scalar(
                                    scr[:, 0:lm], xslot, 1.0, None,
                                    ALU.mult, ALU.add,
                                    accum_out=sumT[b][:, col:col + 1],
                                )
                            scr2 = spool.tile([D, LF], BF16, tag="scr")
                            nc.vector.tensor_scalar(
                                scr2[:, 0:lm], xslot, 1.0, None,
                                ALU.mult, ALU.max,
                                accum_out=maxT[b][:, col:col + 1],
                            )
                            scr3 = spool.tile([D, LF], BF16, tag="scr")
                            nc.vector.scalar_tensor_tensor(
                                out=scr3[:, 0:lm], in0=xslot, scalar=1.0,
                                in1=eps[:, 0:lm],
                                op0=ALU.mult, op1=ALU.mult,
                                accum_out=exT[b][:, col:col + 1],
                            )

            # ---- final stage --------------------------------------------
            with (
                tc.tile_pool(name="fin", bufs=1) as fpool,
                tc.tile_pool(name="fps", bufs=1, space="PSUM") as fps,
            ):
                for b in range(NBLK):
                    rc = fpool.tile([128, 1], F32, name=f"rc{b}")
                    nc.vector.reciprocal(rc[:], counts_bl[b][:])
                    zt = fpool.tile([128, 1], F32, name=f"zt{b}")
                    nc.vector.tensor_sub(zt[:], zcolB[b][:], zcorr_bl[b][:])
                    rz = fpool.tile([128, 1], F32, name=f"rz{b}")
                    nc.vector.reciprocal(rz[:], zt[:])

                    # attn (transposed, unnormalized): outW.T @ exT + outb
                    exbf = fpool.tile([128, 128], BF16, name=f"exbf{b}")
                    nc.vector.tensor_copy(exbf[:], exT[b][:])
                    apT_ps = fps.tile([128, 128], F32, tag="apT")
                    nc.tensor.matmul(apT_ps[:], outW_sb[:], exbf[:],
                                     start=True, stop=True)
                    apT = fpool.tile([128, 128], F32, name=f"apT{b}")
                    nc.scalar.activation(apT[:], apT_ps[:], ACTF.Identity,
                                         bias=outb_sb[:], scale=1.0)

                    # transpose pools to [g, d], normalizing on evacuation
                    mean_gd = fpool.tile([128, 128], F32, name=f"mean_gd{b}")
                    tp = fps.tile([128, 128], F32, tag="tp")
                    nc.tensor.transpose(tp[:], sumT[b][:], ident_sb[:])
                    nc.scalar.activation(mean_gd[:], tp[:], ACTF.Identity,
                                         bias=0.0, scale=rc[:])
                    max_gd = fpool.tile([128, 128], F32, name=f"max_gd{b}")
                    tp2 = fps.tile([128, 128], F32, tag="tp")
                    nc.tensor.transpose(tp2[:], maxT[b][:], ident_sb[:])
                    nc.scalar.activation(max_gd[:], tp2[:], ACTF.Identity,
                                         bias=0.0, scale=1.0)
                    attn_gd = fpool.tile([128, 128], F32, name=f"attn_gd{b}")
                    tp3 = fps.tile([128, 128], F32, tag="tp")
                    nc.tensor.transpose(tp3[:], apT[:], ident_sb[:])
                    nc.scalar.activation(attn_gd[:], tp3[:], ACTF.Identity,
                                         bias=0.0, scale=rz[:])

                    # normalized pools back to [d, g] bf16 for the MLP
                    poolsT_bf = []
                    for nm, gd in (("m", mean_gd), ("x", max_gd),
                                   ("a", attn_gd)):
                        tpp = fps.tile([128, 128], F32, tag="tp")
                        nc.tensor.transpose(tpp[:], gd[:], ident_sb[:])
                        tbf = fpool.tile([128, 128], BF16, name=f"p{nm}T{b}")
                        nc.scalar.activation(tbf[:], tpp[:], ACTF.Identity,
                                             bias=0.0, scale=1.0)
                        poolsT_bf.append(tbf)

                    hid_ps = fps.tile([128, 128], F32, tag="hid")
                    for k in range(3):
                        nc.tensor.matmul(
                            hid_ps[:], selW1_sb[:, k * D:(k + 1) * D],
                            poolsT_bf[k][:],
                            start=(k == 0), stop=(k == 2),
                        )
                    hid_bf = fpool.tile([128, 128], BF16, name=f"hid{b}")
                    nc.scalar.activation(hid_bf[:], hid_ps[:], ACTF.Relu,
                                         bias=selb1_sb[:], scale=1.0)

                    lg_ps = fps.tile([3, 128], F32, tag="lg")
                    nc.tensor.matmul(lg_ps[:], selW2_sb[:], hid_bf[:],
                                     start=True, stop=True)
                    lgT = fpool.tile([3, 128], F32, name=f"lgT{b}")
                    nc.scalar.activation(lgT[:], lg_ps[:], ACTF.Identity,
                                         bias=selb2_sb[:], scale=1.0)

                    lg_ps2 = fps.tile([128, 3], F32, tag="lgt")
                    nc.tensor.transpose(lg_ps2[:], lgT[:], ident_sb[0:3, 0:3])
                    lg = fpool.tile([128, 3], F32, name=f"lg{b}")
                    nc.scalar.activation(lg[:], lg_ps2[:], ACTF.Identity,
                                         bias=0.0, scale=1.0)

                    m3 = fpool.tile([128, 1], F32, name=f"m3{b}")
                    nc.vector.tensor_reduce(m3[:], lg[:], mybir.AxisListType.X,
                                            ALU.max)
                    nm3 = fpool.tile([128, 1], F32, name=f"nm3{b}")
                    nc.vector.tensor_scalar(nm3[:], m3[:], -1.0, None, ALU.mult)
                    ew = fpool.tile([128, 3], F32, name=f"ew{b}")
                    den = fpool.tile([128, 1], F32, name=f"den{b}")
                    nc.scalar.activation(ew[:], lg[:], ACTF.Exp,
                                         bias=nm3[:], scale=1.0,
                                         accum_out=den[:])
                    rden = fpool.tile([128, 1], F32, name=f"rden{b}")
                    nc.vector.reciprocal(rden[:], den[:])
                    w3 = fpool.tile([128, 3], F32, name=f"w3{b}")
                    nc.vector.tensor_scalar(w3[:], ew[:], rden[:], None,
                                            ALU.mult)

                    t1 = fpool.tile([128, 128], F32, name=f"t1{b}")
                    nc.vector.tensor_scalar(t1[:], mean_gd[:], w3[:, 0:1],
                                            None, ALU.mult)
                    t2 = fpool.tile([128, 128], F32, name=f"t2{b}")
                    nc.vector.scalar_tensor_tensor(
                        out=t2[:], in0=max_gd[:], scalar=w3[:, 1:2], in1=t1[:],
                        op0=ALU.mult, op1=ALU.add,
                    )
                    out_sb = fpool.tile([128, D], F32, name=f"out_sb{b}")
                    nc.vector.scalar_tensor_tensor(
                        out=out_sb[:], in0=attn_gd[:],
                        scalar=w3[:, 2:3], in1=t2[:],
                        op0=ALU.mult, op1=ALU.add,
                    )
                    nc.sync.dma_start(
                        outP[b * 128:(b + 1) * 128, :], out_sb[:]
                    )

    nc.compile()
    return nc


# --------------------------------------------------------------------------
# host orchestration
# --------------------------------------------------------------------------

_CACHE = {}


def _shard_meta(batch, G):
    """Per-core starts and the cross-core per-slot trimmed lengths."""
    GPC = G // NCORES
    starts = np.searchsorted(batch, np.arange(G + 1))
    counts = np.diff(starts).astype(np.int64)          # [G]
    percore = counts.reshape(NCORES, GPC)
    lmax = percore.max(axis=0)                          # [GPC]
    lmax = np.minimum(np.maximum((lmax + 3) // 4 * 4, 516), LFIX)
    return starts, counts, lmax


def _prep_core_inputs(x, batch, weights, core, GPC, starts, counts, lmax):
    g0 = core * GPC
    cst = starts[g0:g0 + GPC + 1]
    ccounts = counts[g0:g0 + GPC]
    n_core = cst[-1] - cst[0]
    assert ccounts.max() <= LFIX

    xs = x[cst[0]:cst[-1]]
    local_g = np.repeat(np.arange(GPC), ccounts)
    intra = np.arange(n_core) - np.repeat(cst[:-1] - cst[0], ccounts)
    dest = local_g * LFIX + intra

    NPAD = GPC * LFIX
    xp = np.zeros((NPAD, D), dtype=ml_dtypes.bfloat16)
    xp[dest] = xs.astype(ml_dtypes.bfloat16)
    xTa = np.ascontiguousarray(xp.T)

    (att_W, att_b, att_ctx, out_W, out_b,
     sel_W1, sel_b1, sel_W2, sel_b2) = weights

    attWlo = np.zeros((D, D), np.float32)
    attWlo[:, 0:A] = att_W
    attWhi = np.zeros((D, D), np.float32)
    attWhi[:, A:D] = att_W
    G_ = GROUP
    ctxlo = np.zeros((D, G_ * G_), np.float32)
    ctxhi = np.zeros((D, G_ * G_), np.float32)
    for j in range(G_):
        ctxlo[0:A, j * G_ + j] = att_ctx
        ctxhi[A:D, j * G_ + j] = att_ctx

    s_pad = float(np.tanh(att_b.astype(np.float64)) @
                  att_ctx.astype(np.float64))
    e_pad = math.exp(s_pad)
    # exp covers the full LFIX slot width -> padding contributes e_pad each
    zcorr = ((LFIX - ccounts) * e_pad).astype(np.float32).reshape(GPC, 1)

    return {
        "xT": xTa,
        "attWlo": attWlo.astype(ml_dtypes.bfloat16),
        "attWhi": attWhi.astype(ml_dtypes.bfloat16),
        "attb": np.concatenate([att_b, att_b]).astype(np.float32)
                 .reshape(D, 1),
        "ctxlo": ctxlo.astype(ml_dtypes.bfloat16),
        "ctxhi": ctxhi.astype(ml_dtypes.bfloat16),
        "outW": out_W.astype(ml_dtypes.bfloat16),
        "outb": out_b.astype(np.float32).reshape(D, 1),
        "selW1": sel_W1.astype(ml_dtypes.bfloat16),
        "selb1": sel_b1.astype(np.float32).reshape(D, 1),
        "selW2": sel_W2.astype(ml_dtypes.bfloat16),
        "selb2": sel_b2.astype(np.float32).reshape(3, 1),
        "counts": ccounts.astype(np.float32).reshape(GPC, 1),
        "zcorr": zcorr,
        "ident": np.eye(D, dtype=np.float32),
    }


def _run(x, batch, att_W, att_b, att_ctx, out_W, out_b,
         sel_W1, sel_b1, sel_W2, sel_b2, num_graphs, **spmd_kwargs):
    x = np.asarray(x)
    batch = np.asarray(batch).astype(np.int64)
    G = int(num_graphs)
    assert G == 2048 and x.shape == (1000000, D)
    GPC = G // NCORES

    starts, counts, lmax = _shard_meta(batch, G)

    key = ("nc", LFIX, GPC, tuple(lmax))
    if key not in _CACHE:
        _CACHE[key] = build_nc(LFIX, GPC, GROUP, tuple(lmax))
    nc = _CACHE[key]

    weights = (np.asarray(att_W), np.asarray(att_b), np.asarray(att_ctx),
               np.asarray(out_W), np.asarray(out_b),
               np.asarray(sel_W1), np.asarray(sel_b1),
               np.asarray(sel_W2), np.asarray(sel_b2))

    in_maps = [
        _prep_core_inputs(x, batch, weights, c, GPC, starts, counts, lmax)
        for c in range(NCORES)
    ]

    res = run_bass_kernel_spmd(nc, in_maps, core_ids=list(range(NCORES)),
                               **spmd_kwargs)
    outs = [np.asarray(res.results[c]["out"], dtype=np.float32)
            for c in range(NCORES)]
    return np.concatenate(outs, axis=0), res


def kernel(**inputs):
    return _run(**inputs)[0]



# revision 2
# speedup vs baseline: 5.9541x; 5.9541x over previous
"""AdaptiveGraphPooling v2: dual-layout (fm bf16 + nm fp8), pools on PE.

Per core (GPC=256 slots, LFIX=576):
  fm copy: xT [128, GPC*576] bf16 feature-major (padded slots).
  nm copy: xs [128, GPC*5*128] fp8 node-major staged (640-padded slots,
           window w = 128 nodes; xs[p, w*128+d] = x_nm[w*128+p, d]).

  Per supergroup of 32 slots:
    - h: col-tiled matmul pairs (attW [128, 64], outs at psum 0:64/64:128)
    - tanh: ACT on [128, 1024] psum spans -> thc bf16 (2 nodes per col)
    - scores: one-hot ctx matmuls, 4x col-tiled by subgroup (M=32),
      static runs mapping slots to thc cols -> sps psum [128 rows, 576]
    - exp: ACT -> E_sb [128, 576] bf16, accum z32
    - E transpose: PE [8, 128] blocks -> etile psum bf16 -> DVE strided
      evac into rhs_t fp8 (e at odd cols, ones at even)
    - pools: 160 fp8 matmuls (lhsT = 128-node window, rhs = [1|e]) accum
      into a resident psum bank [128, 512] = (sum, exsum) per slot
    - max: DVE tensor_scalar 4x accum per slot from fm
  Final: per 128-slot block, normalize / selector MLP / mix (as v1).
"""

import math
import numpy as np
import ml_dtypes

from concourse import bass, bacc, mybir
from concourse import tile as tile_mod
from concourse.bass_utils import run_bass_kernel_spmd

BF16 = mybir.dt.bfloat16
F32 = mybir.dt.float32
FP8 = mybir.dt.float8e4
FP8NP = ml_dtypes.float8_e4m3
ALU = mybir.AluOpType
ACTF = mybir.ActivationFunctionType

D = 128
A = 64
NCORES = 8
LFIX = 576
LNM = 640
WPS = LNM // 128          # windows per slot (5)
SG = 32                   # slots per supergroup
GPC = 256
NSG = GPC // SG
NPADF = GPC * LFIX        # 147456
NW = GPC * WPS            # 1280 windows
NBLK = GPC // 128


def _score_runs():
    """Static (slot_local, runs) where each run maps a slot's node segment to
    thc coords: (half, col0, l0, ln). thc col = t*1024 + q*512 + (n%512),
    half = ((n%2048)//512) & 1, q = ((n%2048)//512) >> 1, t = n//2048."""
    out = []
    for j in range(SG):
        n0 = j * LFIX
        runs = []
        n = n0
        while n < n0 + LFIX:
            nxt = min((n // 512 + 1) * 512, n0 + LFIX)
            l0 = n - n0
            if l0 < 512 < (nxt - n0):
                nxt = n0 + 512
            b = (n % 2048) // 512
            half = b & 1
            col0 = (n // 2048) * 1024 + (b >> 1) * 512 + (n % 512)
            runs.append((half, col0, l0, nxt - n))
            n = nxt
        out.append(runs)
    return out


def build_nc_v2(reps=1):
    nc = bacc.Bacc(None, target_bir_lowering=False, debug=False)

    xT = nc.declare_dram_parameter("xT", [D, NPADF], BF16, isOutput=False)
    xs = nc.declare_dram_parameter("xs", [D, NW * D], FP8, isOutput=False)
    attW = nc.declare_dram_parameter("attW", [D, A], BF16, isOutput=False)
    attb2 = nc.declare_dram_parameter("attb2", [D, 1], F32, isOutput=False)
    ctxoh = nc.declare_dram_parameter("ctxoh", [D, 16 * 32], BF16,
                                      isOutput=False)
    ident8 = nc.declare_dram_parameter("ident8", [D, 8], BF16, isOutput=False)
    outW = nc.declare_dram_parameter("outW", [D, D], BF16, isOutput=False)
    outb = nc.declare_dram_parameter("outb", [D, 1], F32, isOutput=False)
    selW1 = nc.declare_dram_parameter("selW1", [3 * D, D], BF16,
                                      isOutput=False)
    selb1 = nc.declare_dram_parameter("selb1", [D, 1], F32, isOutput=False)
    selW2 = nc.declare_dram_parameter("selW2", [D, 3], BF16, isOutput=False)
    selb2 = nc.declare_dram_parameter("selb2", [3, 1], F32, isOutput=False)
    countsP = nc.declare_dram_parameter("counts", [GPC, 1], F32,
                                        isOutput=False)
    zcorrP = nc.declare_dram_parameter("zcorr", [GPC, 1], F32, isOutput=False)
    identP = nc.declare_dram_parameter("ident", [D, D], F32, isOutput=False)
    outP = nc.declare_dram_parameter("out", [GPC, D], F32, isOutput=True)

    runs_by_slot = _score_runs()

    with tile_mod.TileContext(nc) as tc:
        with (
            tc.tile_pool(name="pools_ps", bufs=1, space="PSUM") as pools_ps,
            tc.tile_pool(name="const", bufs=1) as cpool,
        ):
            pools_bank = pools_ps.tile([D, 512], F32, name="pools_bank")

            attW_sb = cpool.tile([D, A], BF16)
            nc.sync.dma_start(attW_sb[:], attW[:])
            attb2_sb = cpool.tile([D, 1], F32)
            nc.sync.dma_start(attb2_sb[:], attb2[:])
            ctxoh_sb = cpool.tile([D, 16 * 32], BF16)
            nc.sync.dma_start(ctxoh_sb[:], ctxoh[:])
            ident8_sb = cpool.tile([D, 8], BF16)
            nc.sync.dma_start(ident8_sb[:], ident8[:])
            outW_sb = cpool.tile([D, D], BF16)
            nc.sync.dma_start(outW_sb[:], outW[:])
            outb_sb = cpool.tile([D, 1], F32)
            nc.sync.dma_start(outb_sb[:], outb[:])
            selW1_sb = cpool.tile([D, 3 * D], BF16)
            for k in range(3):
                nc.sync.dma_start(
                    selW1_sb[:, k * D:(k + 1) * D], selW1[k * D:(k + 1) * D, :]
                )
            selb1_sb = cpool.tile([D, 1], F32)
            nc.sync.dma_start(selb1_sb[:], selb1[:])
            selW2_sb = cpool.tile([D, 3], BF16)
            nc.sync.dma_start(selW2_sb[:], selW2[:])
            selb2_sb = cpool.tile([3, 1], F32)
            nc.sync.dma_start(selb2_sb[:], selb2[:])
            ident_sb = cpool.tile([D, D], F32)
            nc.sync.dma_start(ident_sb[:], identP[:])

            counts_bl, zcorr_bl, maxT, zcolB = [], [], [], []
            for b in range(NBLK):
                cb = cpool.tile([128, 1], F32, name=f"counts_b{b}")
                nc.sync.dma_start(cb[:], countsP[b * 128:(b + 1) * 128, :])
                counts_bl.append(cb)
                zb = cpool.tile([128, 1], F32, name=f"zcorr_b{b}")
                nc.sync.dma_start(zb[:], zcorrP[b * 128:(b + 1) * 128, :])
                zcorr_bl.append(zb)
                maxT.append(cpool.tile([128, 128], F32, name=f"maxT{b}"))
                zcolB.append(cpool.tile([128, 1], F32, name=f"zcol{b}"))

            with (
                tc.tile_pool(name="xf", bufs=2) as xfpool,
                tc.tile_pool(name="xn", bufs=2) as xnpool,
                tc.tile_pool(name="thc", bufs=2) as thcpool,
                tc.tile_pool(name="esb", bufs=2) as epool,
                tc.tile_pool(name="rhs", bufs=2) as rpool,
                tc.tile_pool(name="scr", bufs=6) as spool,
                tc.tile_pool(name="zt", bufs=2) as zpool,
                tc.tile_pool(name="thp", bufs=2, space="PSUM") as thps,
                tc.tile_pool(name="sps", bufs=1, space="PSUM") as sps_pool,
                tc.tile_pool(name="eps", bufs=1, space="PSUM") as eps_pool,
            ):
              for rep in range(reps):
                for sg in range(NSG):
                    xt = xfpool.tile([D, SG * LFIX], BF16, tag="xt",
                                     name=f"xt{sg}")
                    nc.sync.dma_start(
                        xt[:], xT[:, sg * SG * LFIX:(sg + 1) * SG * LFIX])
                    xs_t = xnpool.tile([D, SG * WPS * D], FP8, tag="xs",
                                       name=f"xs{sg}")
                    nc.sync.dma_start(
                        xs_t[:],
                        xs[:, sg * SG * WPS * D:(sg + 1) * SG * WPS * D])

                    # --- h + tanh (paired halves) --------------------------
                    thc = thcpool.tile([D, SG * LFIX // 2], BF16, tag="thc",
                                       name=f"thc{sg}")
                    for t in range(SG * LFIX // 2048):
                        thp = thps.tile([D, 1024], F32, tag="thp")
                        for q in (0, 1):
                            base = t * 2048 + q * 1024
                            nc.tensor.matmul(
                                thp[0:64, q * 512:q * 512 + 512],
                                attW_sb[:], xt[:, base:base + 512],
                                start=True, stop=True,
                                skip_group_check=True,
                            )
                            nc.tensor.matmul(
                                thp[64:128, q * 512:q * 512 + 512],
                                attW_sb[:], xt[:, base + 512:base + 1024],
                                start=True, stop=True,
                                skip_group_check=True,
                            )
                        nc.scalar.activation(
                            thc[:, t * 1024:(t + 1) * 1024], thp[:],
                            ACTF.Tanh, bias=attb2_sb[:], scale=1.0,
                        )

                    # --- scores (one-hot, col-tiled by subgroup) -----------
                    sps512 = sps_pool.tile([D, 512], F32, tag="sps512")
                    sps64 = sps_pool.tile([D, 64], F32, tag="sps64")
                    seen = {}
                    last = {}
                    for j in range(SG):
                        c, r = j // 8, j % 8
                        for ri, (half, col0, l0, ln) in \
                                enumerate(runs_by_slot[j]):
                            tile512 = l0 < 512
                            last[(c, tile512)] = (j, ri)
                    for j in range(SG):
                        c, r = j // 8, j % 8
                        for ri, (half, col0, l0, ln) in \
                                enumerate(runs_by_slot[j]):
                            tile512 = l0 < 512
                            v = half * 8 + r
                            # full-K lhsT: inactive half rows are zero, so
                            # the other half's thc contributes nothing.
                            lhs = ctxoh_sb[:, v * 32:(v + 1) * 32]
                            dst = (sps512[32 * c:32 * c + 32, l0:l0 + ln]
                                   if tile512 else
                                   sps64[32 * c:32 * c + 32,
                                         l0 - 512:l0 - 512 + ln])
                            key = (c, tile512)
                            nc.tensor.matmul(
                                dst, lhs, thc[:, col0:col0 + ln],
                                start=key not in seen,
                                stop=last[key] == (j, ri),
                                tile_position=(0, 32 * c),
                                skip_group_check=True,
                            )
                            seen[key] = True

                    # --- exp + z ------------------------------------------
                    E_sb = epool.tile([D, LNM], BF16, tag="esb",
                                      name=f"E{sg}")
                    z32a = zpool.tile([D, 1], F32, tag="za")
                    z32b = zpool.tile([D, 1], F32, tag="zb")
                    nc.scalar.activation(E_sb[:, 0:512], sps512[:], ACTF.Exp,
                                         accum_out=z32a[:])
                    nc.scalar.activation(E_sb[:, 512:576], sps64[:], ACTF.Exp,
                                         accum_out=z32b[:])
                    nc.vector.memset(E_sb[:, LFIX:LNM], 0.0)
                    z32 = zpool.tile([D, 1], F32, tag="z")
                    nc.vector.tensor_add(z32[:], z32a[:], z32b[:])
                    b0 = sg // 4
                    for c in range(4):
                        nc.sync.dma_start(
                            zcolB[b0][(sg % 4) * 32 + c * 8:
                                      (sg % 4) * 32 + c * 8 + 8, 0:1],
                            z32[32 * c:32 * c + 8, 0:1],
                        )

                    # --- E transpose -> rhs ones[0:160] | e[160:320] -------
                    # etile col layout (c, k, r): transpose (c, k) emits its
                    # 8 slot-rows as 8 consecutive cols.
                    NWSG = SG * WPS
                    # stage subgroup rows to partition base 0 (transpose-mode
                    # matmuls only work at row base 0 on HW)
                    ecopy = epool.tile([8, 4 * LNM], BF16, tag="ecopy")
                    for c in range(4):
                        nc.sync.dma_start(
                            ecopy[0:8, c * LNM:(c + 1) * LNM],
                            E_sb[32 * c:32 * c + 8, :])
                    etile = eps_pool.tile([D, NWSG], BF16, tag="et")
                    for c in range(4):
                        for k in range(WPS):
                            nc.tensor.transpose(
                                etile[:, (c * WPS + k) * 8:
                                      (c * WPS + k) * 8 + 8],
                                ecopy[0:8, c * LNM + 128 * k:
                                      c * LNM + 128 * k + 128],
                                ident8_sb[0:8, :],
                            )
                    rhs_t = rpool.tile([D, 2 * NWSG], FP8, tag="rhs")
                    nc.vector.memset(rhs_t[:, 0:NWSG], 1.0)
                    nc.vector.tensor_copy(rhs_t[:, NWSG:2 * NWSG], etile[:])

                    # --- pools matmuls ------------------------------------
                    # window (j=c*8+r, k): lhsT at xs col block j*WPS+k,
                    # e col idx = c*40 + k*8 + r; rhs = (ones[idx], e[idx]).
                    for j in range(SG):
                        c, r = j // 8, j % 8
                        s2 = (sg * SG + j) * 2
                        for k in range(WPS):
                            wi = j * WPS + k
                            idx = c * 8 * WPS + k * 8 + r
                            nc.tensor.matmul(
                                pools_bank[:, s2:s2 + 2],
                                xs_t[:, wi * D:(wi + 1) * D],
                                rhs_t[:, idx:idx + NWSG + 1:NWSG],
                                start=(k == 0), stop=(k == WPS - 1),
                            )

                    # --- max ----------------------------------------------
                    for j in range(SG):
                        g = sg * SG + j
                        b, col = g // 128, g % 128
                        scr = spool.tile([D, LFIX], BF16, tag="scr")
                        nc.vector.tensor_scalar(
                            scr[:], xt[:, j * LFIX:(j + 1) * LFIX], 1.0,
                            None, ALU.mult, ALU.max,
                            accum_out=maxT[b][:, col:col + 1],
                        )

            # ---- final stage --------------------------------------------
            with (
                tc.tile_pool(name="fin", bufs=1) as fpool,
                tc.tile_pool(name="fps", bufs=1, space="PSUM") as fps,
            ):
                for b in range(NBLK):
                    sumT = fpool.tile([128, 128], F32, name=f"sumT{b}")
                    nc.vector.tensor_copy(
                        sumT[:], pools_bank[:, b * 256:b * 256 + 256:2])
                    exT = fpool.tile([128, 128], F32, name=f"exT{b}")
                    nc.vector.tensor_copy(
                        exT[:], pools_bank[:, b * 256 + 1:b * 256 + 256:2])

                    rc = fpool.tile([128, 1], F32, name=f"rc{b}")
                    nc.vector.reciprocal(rc[:], counts_bl[b][:])
                    zt = fpool.tile([128, 1], F32, name=f"zt{b}")
                    nc.vector.tensor_sub(zt[:], zcolB[b][:], zcorr_bl[b][:])
                    rz = fpool.tile([128, 1], F32, name=f"rz{b}")
                    nc.vector.reciprocal(rz[:], zt[:])

                    exbf = fpool.tile([128, 128], BF16, name=f"exbf{b}")
                    nc.vector.tensor_copy(exbf[:], exT[:])
                    apT_ps = fps.tile([128, 128], F32, tag="apT")
                    nc.tensor.matmul(apT_ps[:], outW_sb[:], exbf[:],
                                     start=True, stop=True)
                    apT = fpool.tile([128, 128], F32, name=f"apT{b}")
                    nc.scalar.activation(apT[:], apT_ps[:], ACTF.Identity,
                                         bias=outb_sb[:], scale=1.0)

                    mean_gd = fpool.tile([128, 128], F32, name=f"mean_gd{b}")
                    tp = fps.tile([128, 128], F32, tag="tp")
                    nc.tensor.transpose(tp[:], sumT[:], ident_sb[:])
                    nc.scalar.activation(mean_gd[:], tp[:], ACTF.Identity,
                                         bias=0.0, scale=rc[:])
                    max_gd = fpool.tile([128, 128], F32, name=f"max_gd{b}")
                    tp2 = fps.tile([128, 128], F32, tag="tp")
                    nc.tensor.transpose(tp2[:], maxT[b][:], ident_sb[:])
                    nc.scalar.activation(max_gd[:], tp2[:], ACTF.Identity,
                                         bias=0.0, scale=1.0)
                    attn_gd = fpool.tile([128, 128], F32, name=f"attn_gd{b}")
                    tp3 = fps.tile([128, 128], F32, tag="tp")
                    nc.tensor.transpose(tp3[:], apT[:], ident_sb[:])
                    nc.scalar.activation(attn_gd[:], tp3[:], ACTF.Identity,
                                         bias=0.0, scale=rz[:])

                    poolsT_bf = []
                    for nm, gd in (("m", mean_gd), ("x", max_gd),
                                   ("a", attn_gd)):
                        tpp = fps.tile([128, 128], F32, tag="tp")
                        nc.tensor.transpose(tpp[:], gd[:], ident_sb[:])
                        tbf = fpool.tile([128, 128], BF16, name=f"p{nm}T{b}")
                        nc.scalar.activation(tbf[:], tpp[:], ACTF.Identity,
                                             bias=0.0, scale=1.0)
                        poolsT_bf.append(tbf)

                    hid_ps = fps.tile([128, 128], F32, tag="hid")
                    for k in range(3):
                        nc.tensor.matmul(
                            hid_ps[:], selW1_sb[:, k * D:(k + 1) * D],
                            poolsT_bf[k][:],
                            start=(k == 0), stop=(k == 2),
                        )
                    hid_bf = fpool.tile([128, 128], BF16, name=f"hid{b}")
                    nc.scalar.activation(hid_bf[:], hid_ps[:], ACTF.Relu,
                                         bias=selb1_sb[:], scale=1.0)

                    lg_ps = fps.tile([3, 128], F32, tag="lg")
                    nc.tensor.matmul(lg_ps[:], selW2_sb[:], hid_bf[:],
                                     start=True, stop=True)
                    lgT = fpool.tile([3, 128], F32, name=f"lgT{b}")
                    nc.scalar.activation(lgT[:], lg_ps[:], ACTF.Identity,
                                         bias=selb2_sb[:], scale=1.0)

                    lg_ps2 = fps.tile([128, 3], F32, tag="lgt")
                    nc.tensor.transpose(lg_ps2[:], lgT[:], ident_sb[0:3, 0:3])
                    lg = fpool.tile([128, 3], F32, name=f"lg{b}")
                    nc.scalar.activation(lg[:], lg_ps2[:], ACTF.Identity,
                                         bias=0.0, scale=1.0)

                    m3 = fpool.tile([128, 1], F32, name=f"m3{b}")
                    nc.vector.tensor_reduce(m3[:], lg[:], mybir.AxisListType.X,
                                            ALU.max)
                    nm3 = fpool.tile([128, 1], F32, name=f"nm3{b}")
                    nc.vector.tensor_scalar(nm3[:], m3[:], -1.0, None,
                                            ALU.mult)
                    ew = fpool.tile([128, 3], F32, name=f"ew{b}")
                    den = fpool.tile([128, 1], F32, name=f"den{b}")
                    nc.scalar.activation(ew[:], lg[:], ACTF.Exp,
                                         bias=nm3[:], scale=1.0,
                                         accum_out=den[:])
                    rden = fpool.tile([128, 1], F32, name=f"rden{b}")
                    nc.vector.reciprocal(rden[:], den[:])
                    w3 = fpool.tile([128, 3], F32, name=f"w3{b}")
                    nc.vector.tensor_scalar(w3[:], ew[:], rden[:], None,
                                            ALU.mult)

                    t1 = fpool.tile([128, 128], F32, name=f"t1{b}")
                    nc.vector.tensor_scalar(t1[:], mean_gd[:], w3[:, 0:1],
                                            None, ALU.mult)
                    t2 = fpool.tile([128, 128], F32, name=f"t2{b}")
                    nc.vector.scalar_tensor_tensor(
                        out=t2[:], in0=max_gd[:], scalar=w3[:, 1:2], in1=t1[:],
                        op0=ALU.mult, op1=ALU.add,
                    )
                    out_sb = fpool.tile([128, D], F32, name=f"out_sb{b}")
                    nc.vector.scalar_tensor_tensor(
                        out=out_sb[:], in0=attn_gd[:],
                        scalar=w3[:, 2:3], in1=t2[:],
                        op0=ALU.mult, op1=ALU.add,
                    )
                    nc.sync.dma_start(
                        outP[b * 128:(b + 1) * 128, :], out_sb[:]
                    )

    nc.compile()
    return nc


# --------------------------------------------------------------------------
# host orchestration
# --------------------------------------------------------------------------

_CACHE = {}


def _shard_meta(batch, G):
    starts = np.searchsorted(batch, np.arange(G + 1))
    counts = np.diff(starts).astype(np.int64)
    return starts, counts


def _prep_core_inputs(x, batch, weights, core, starts, counts):
    g0 = core * GPC
    cst = starts[g0:g0 + GPC + 1]
    ccounts = counts[g0:g0 + GPC]
    n_core = cst[-1] - cst[0]
    assert ccounts.max() <= LFIX, ccounts.max()

    xs_nodes = x[cst[0]:cst[-1]]
    local_g = np.repeat(np.arange(GPC), ccounts)
    intra = np.arange(n_core) - np.repeat(cst[:-1] - cst[0], ccounts)

    xbf = xs_nodes.astype(ml_dtypes.bfloat16)

    # fm copy
    destf = local_g * LFIX + intra
    xpf = np.zeros((GPC * LFIX, D), dtype=ml_dtypes.bfloat16)
    xpf[destf] = xbf
    xTa = np.ascontiguousarray(xpf.T)

    # nm staged copy (640-pad)
    destn = local_g * LNM + intra
    xpn = np.zeros((GPC * LNM, D), dtype=FP8NP)
    xpn[destn] = xs_nodes.astype(FP8NP)
    xsa = np.ascontiguousarray(
        xpn.reshape(NW, 128, D).transpose(1, 0, 2).reshape(128, NW * D))

    (att_W, att_b, att_ctx, out_W, out_b,
     sel_W1, sel_b1, sel_W2, sel_b2) = weights

    ctxoh = np.zeros((D, 16 * 32), np.float32)
    for half in range(2):
        for r in range(8):
            v = half * 8 + r
            ctxoh[half * 64:(half + 1) * 64, v * 32 + r] = att_ctx

    s_pad = float(np.tanh(att_b.astype(np.float64)) @
                  att_ctx.astype(np.float64))
    e_pad = math.exp(s_pad)
    zcorr = ((LFIX - ccounts) * e_pad).astype(np.float32).reshape(GPC, 1)

    return {
        "xT": xTa,
        "xs": xsa,
        "attW": att_W.astype(ml_dtypes.bfloat16),
        "attb2": np.concatenate([att_b, att_b]).astype(np.float32)
                  .reshape(D, 1),
        "ctxoh": ctxoh.astype(ml_dtypes.bfloat16),
        "ident8": np.tile(np.eye(8, dtype=np.float32), (16, 1))
                   .astype(ml_dtypes.bfloat16),
        "outW": out_W.astype(ml_dtypes.bfloat16),
        "outb": out_b.astype(np.float32).reshape(D, 1),
        "selW1": sel_W1.astype(ml_dtypes.bfloat16),
        "selb1": sel_b1.astype(np.float32).reshape(D, 1),
        "selW2": sel_W2.astype(ml_dtypes.bfloat16),
        "selb2": sel_b2.astype(np.float32).reshape(3, 1),
        "counts": ccounts.astype(np.float32).reshape(GPC, 1),
        "zcorr": zcorr,
        "ident": np.eye(D, dtype=np.float32),
    }


def _run(x, batch, att_W, att_b, att_ctx, out_W, out_b,
         sel_W1, sel_b1, sel_W2, sel_b2, num_graphs, **spmd_kwargs):
    x = np.asarray(x)
    batch = np.asarray(batch).astype(np.int64)
    G = int(num_graphs)
    assert G == 2048 and x.shape == (1000000, D)

    starts, counts = _shard_meta(batch, G)

    if "nc" not in _CACHE:
        _CACHE["nc"] = build_nc_v2()
    nc = _CACHE["nc"]

    weights = (np.asarray(att_W), np.asarray(att_b), np.asarray(att_ctx),
               np.asarray(out_W), np.asarray(out_b),
               np.asarray(sel_W1), np.asarray(sel_b1),
               np.asarray(sel_W2), np.asarray(sel_b2))

    in_maps = [
        _prep_core_inputs(x, batch, weights, c, starts, counts)
        for c in range(NCORES)
    ]

    res = run_bass_kernel_spmd(nc, in_maps, core_ids=list(range(NCORES)),
                               **spmd_kwargs)
    outs = [np.asarray(res.results[c]["out"], dtype=np.float32)
            for c in range(NCORES)]
    return np.concatenate(outs, axis=0), res


def kernel(**inputs):
    return _run(**inputs)[0]


# revision 4
# speedup vs baseline: 7.1745x; 1.2050x over previous
"""AdaptiveGraphPooling v2: dual-layout (fm bf16 + nm fp8), pools on PE.

Per core (GPC=256 slots, LFIX=576):
  fm copy: xT [128, GPC*576] bf16 feature-major (padded slots).
  nm copy: xs [128, GPC*5*128] fp8 node-major staged (640-padded slots,
           window w = 128 nodes; xs[p, w*128+d] = x_nm[w*128+p, d]).

  Per supergroup of 32 slots:
    - h: col-tiled matmul pairs (attW [128, 64], outs at psum 0:64/64:128)
    - tanh: ACT on [128, 1024] psum spans -> thc bf16 (2 nodes per col)
    - scores: one-hot ctx matmuls, 4x col-tiled by subgroup (M=32),
      static runs mapping slots to thc cols -> sps psum [128 rows, 576]
    - exp: ACT -> E_sb [128, 576] bf16, accum z32
    - E transpose: PE [8, 128] blocks -> etile psum bf16 -> DVE strided
      evac into rhs_t fp8 (e at odd cols, ones at even)
    - pools: 160 fp8 matmuls (lhsT = 128-node window, rhs = [1|e]) accum
      into a resident psum bank [128, 512] = (sum, exsum) per slot
    - max: DVE tensor_scalar 4x accum per slot from fm
  Final: per 128-slot block, normalize / selector MLP / mix (as v1).
"""

import math
import numpy as np
import ml_dtypes

from concourse import bass, bacc, mybir
from concourse import tile as tile_mod
from concourse.bass_utils import run_bass_kernel_spmd

BF16 = mybir.dt.bfloat16
F32 = mybir.dt.float32
FP8 = mybir.dt.float8e4
FP8NP = ml_dtypes.float8_e4m3
ALU = mybir.AluOpType
ACTF = mybir.ActivationFunctionType

D = 128
A = 64
NCORES = 8
LFIX0 = 576               # default slot width (fm); raised if a graph is larger
SG = 32                   # slots per supergroup
GPC = 256
NSG = GPC // SG
NBLK = GPC // 128


def _derived(LFIX):
    assert LFIX % 64 == 0 and 512 < LFIX <= 1024
    LNM = ((LFIX + 127) // 128) * 128
    WPS = LNM // 128
    NPADF = GPC * LFIX
    NW = GPC * WPS
    return LNM, WPS, NPADF, NW


def _score_runs(LFIX):
    """Static (slot_local, runs) where each run maps a slot's node segment to
    thc coords: (half, col0, l0, ln). thc col = t*1024 + q*512 + (n%512),
    half = ((n%2048)//512) & 1, q = ((n%2048)//512) >> 1, t = n//2048."""
    out = []
    for j in range(SG):
        n0 = j * LFIX
        runs = []
        n = n0
        while n < n0 + LFIX:
            nxt = min((n // 512 + 1) * 512, n0 + LFIX)
            l0 = n - n0
            if l0 < 512 < (nxt - n0):
                nxt = n0 + 512
            b = (n % 2048) // 512
            half = b & 1
            col0 = (n // 2048) * 1024 + (b >> 1) * 512 + (n % 512)
            runs.append((half, col0, l0, nxt - n))
            n = nxt
        out.append(runs)
    return out


def build_nc_v2(reps=1, LFIX=LFIX0):
    LNM, WPS, NPADF, NW = _derived(LFIX)
    nc = bacc.Bacc(None, target_bir_lowering=False, debug=False)

    xT = nc.declare_dram_parameter("xT", [D, NPADF], BF16, isOutput=False)
    xs = nc.declare_dram_parameter("xs", [D, NW * D], FP8, isOutput=False)
    attW = nc.declare_dram_parameter("attW", [D, A], BF16, isOutput=False)
    attb2 = nc.declare_dram_parameter("attb2", [D, 1], F32, isOutput=False)
    ctxoh = nc.declare_dram_parameter("ctxoh", [D, 16 * 32], BF16,
                                      isOutput=False)
    ident8 = nc.declare_dram_parameter("ident8", [D, 8], BF16, isOutput=False)
    outW = nc.declare_dram_parameter("outW", [D, D], BF16, isOutput=False)
    outb = nc.declare_dram_parameter("outb", [D, 1], F32, isOutput=False)
    selW1 = nc.declare_dram_parameter("selW1", [3 * D, D], BF16,
                                      isOutput=False)
    selb1 = nc.declare_dram_parameter("selb1", [D, 1], F32, isOutput=False)
    selW2 = nc.declare_dram_parameter("selW2", [D, 3], BF16, isOutput=False)
    selb2 = nc.declare_dram_parameter("selb2", [3, 1], F32, isOutput=False)
    countsP = nc.declare_dram_parameter("counts", [GPC, 1], F32,
                                        isOutput=False)
    zcorrP = nc.declare_dram_parameter("zcorr", [GPC, 1], F32, isOutput=False)
    identP = nc.declare_dram_parameter("ident", [D, D], F32, isOutput=False)
    outP = nc.declare_dram_parameter("out", [GPC, D], F32, isOutput=True)

    runs_by_slot = _score_runs(LFIX)

    with tile_mod.TileContext(nc) as tc:
        with (
            tc.tile_pool(name="pools_ps", bufs=1, space="PSUM") as pools_ps,
            tc.tile_pool(name="const", bufs=1) as cpool,
        ):
            pools_bank = pools_ps.tile([D, 512], F32, name="pools_bank")

            attW_sb = cpool.tile([D, A], BF16)
            nc.sync.dma_start(attW_sb[:], attW[:])
            attb2_sb = cpool.tile([D, 1], F32)
            nc.sync.dma_start(attb2_sb[:], attb2[:])
            ctxoh_sb = cpool.tile([D, 16 * 32], BF16)
            nc.sync.dma_start(ctxoh_sb[:], ctxoh[:])
            ident8_sb = cpool.tile([D, 8], BF16)
            nc.sync.dma_start(ident8_sb[:], ident8[:])
            outW_sb = cpool.tile([D, D], BF16)
            nc.sync.dma_start(outW_sb[:], outW[:])
            outb_sb = cpool.tile([D, 1], F32)
            nc.sync.dma_start(outb_sb[:], outb[:])
            selW1_sb = cpool.tile([D, 3 * D], BF16)
            for k in range(3):
                nc.sync.dma_start(
                    selW1_sb[:, k * D:(k + 1) * D], selW1[k * D:(k + 1) * D, :]
                )
            selb1_sb = cpool.tile([D, 1], F32)
            nc.sync.dma_start(selb1_sb[:], selb1[:])
            selW2_sb = cpool.tile([D, 3], BF16)
            nc.sync.dma_start(selW2_sb[:], selW2[:])
            selb2_sb = cpool.tile([3, 1], F32)
            nc.sync.dma_start(selb2_sb[:], selb2[:])
            ident_sb = cpool.tile([D, D], F32)
            nc.sync.dma_start(ident_sb[:], identP[:])

            counts_bl, zcorr_bl, maxT, zcolB = [], [], [], []
            for b in range(NBLK):
                cb = cpool.tile([128, 1], F32, name=f"counts_b{b}")
                nc.sync.dma_start(cb[:], countsP[b * 128:(b + 1) * 128, :])
                counts_bl.append(cb)
                zb = cpool.tile([128, 1], F32, name=f"zcorr_b{b}")
                nc.sync.dma_start(zb[:], zcorrP[b * 128:(b + 1) * 128, :])
                zcorr_bl.append(zb)
                maxT.append(cpool.tile([128, 128], F32, name=f"maxT{b}"))
                zcolB.append(cpool.tile([128, 1], F32, name=f"zcol{b}"))

            with (
                tc.tile_pool(name="xf", bufs=2) as xfpool,
                tc.tile_pool(name="xn", bufs=2) as xnpool,
                tc.tile_pool(name="thc", bufs=2) as thcpool,
                tc.tile_pool(name="esb", bufs=2) as epool,
                tc.tile_pool(name="rhs", bufs=2) as rpool,
                tc.tile_pool(name="scr", bufs=6) as spool,
                tc.tile_pool(name="zt", bufs=2) as zpool,
                tc.tile_pool(name="thp", bufs=2, space="PSUM") as thps,
                tc.tile_pool(name="sps", bufs=1, space="PSUM") as sps_pool,
                tc.tile_pool(name="eps", bufs=1, space="PSUM") as eps_pool,
            ):
              for rep in range(reps):
                for sg in range(NSG):
                    xt = xfpool.tile([D, SG * LFIX], BF16, tag="xt",
                                     name=f"xt{sg}")
                    nc.sync.dma_start(
                        xt[:], xT[:, sg * SG * LFIX:(sg + 1) * SG * LFIX])
                    xs_t = xnpool.tile([D, SG * WPS * D], FP8, tag="xs",
                                       name=f"xs{sg}")
                    nc.sync.dma_start(
                        xs_t[:],
                        xs[:, sg * SG * WPS * D:(sg + 1) * SG * WPS * D])

                    # --- h + tanh (paired halves) --------------------------
                    thc = thcpool.tile([D, SG * LFIX // 2], BF16, tag="thc",
                                       name=f"thc{sg}")
                    for t in range(SG * LFIX // 2048):
                        thp = thps.tile([D, 1024], F32, tag="thp")
                        for q in (0, 1):
                            base = t * 2048 + q * 1024
                            nc.tensor.matmul(
                                thp[0:64, q * 512:q * 512 + 512],
                                attW_sb[:], xt[:, base:base + 512],
                                start=True, stop=True,
                                skip_group_check=True,
                            )
                            nc.tensor.matmul(
                                thp[64:128, q * 512:q * 512 + 512],
                                attW_sb[:], xt[:, base + 512:base + 1024],
                                start=True, stop=True,
                                skip_group_check=True,
                            )
                        nc.scalar.activation(
                            thc[:, t * 1024:(t + 1) * 1024], thp[:],
                            ACTF.Tanh, bias=attb2_sb[:], scale=1.0,
                        )

                    # --- scores (one-hot, col-tiled by subgroup) -----------
                    sps512 = sps_pool.tile([D, 512], F32, tag="sps512")
                    sps64 = sps_pool.tile([D, LFIX - 512], F32, tag="sps64")
                    seen = {}
                    last = {}
                    for j in range(SG):
                        c, r = j // 8, j % 8
                        for ri, (half, col0, l0, ln) in \
                                enumerate(runs_by_slot[j]):
                            tile512 = l0 < 512
                            last[(c, tile512)] = (j, ri)
                    for j in range(SG):
                        c, r = j // 8, j % 8
                        for ri, (half, col0, l0, ln) in \
                                enumerate(runs_by_slot[j]):
                            tile512 = l0 < 512
                            v = half * 8 + r
                            # full-K lhsT: inactive half rows are zero, so
                            # the other half's thc contributes nothing.
                            lhs = ctxoh_sb[:, v * 32:(v + 1) * 32]
                            dst = (sps512[32 * c:32 * c + 32, l0:l0 + ln]
                                   if tile512 else
                                   sps64[32 * c:32 * c + 32,
                                         l0 - 512:l0 - 512 + ln])
                            key = (c, tile512)
                            nc.tensor.matmul(
                                dst, lhs, thc[:, col0:col0 + ln],
                                start=key not in seen,
                                stop=last[key] == (j, ri),
                                tile_position=(0, 32 * c),
                                skip_group_check=True,
                            )
                            seen[key] = True

                    # --- exp + z ------------------------------------------
                    E_sb = epool.tile([D, LNM], BF16, tag="esb",
                                      name=f"E{sg}")
                    z32a = zpool.tile([D, 1], F32, tag="za")
                    z32b = zpool.tile([D, 1], F32, tag="zb")
                    nc.scalar.activation(E_sb[:, 0:512], sps512[:], ACTF.Exp,
                                         accum_out=z32a[:])
                    nc.scalar.activation(E_sb[:, 512:LFIX], sps64[:],
                                         ACTF.Exp, accum_out=z32b[:])
                    if LNM > LFIX:
                        nc.vector.memset(E_sb[:, LFIX:LNM], 0.0)
                    z32 = zpool.tile([D, 1], F32, tag="z")
                    nc.vector.tensor_add(z32[:], z32a[:], z32b[:])
                    b0 = sg // 4
                    for c in range(4):
                        nc.sync.dma_start(
                            zcolB[b0][(sg % 4) * 32 + c * 8:
                                      (sg % 4) * 32 + c * 8 + 8, 0:1],
                            z32[32 * c:32 * c + 8, 0:1],
                        )

                    # --- E transpose -> rhs ones[0:160] | e[160:320] -------
                    # etile col layout (c, k, r): transpose (c, k) emits its
                    # 8 slot-rows as 8 consecutive cols.
                    NWSG = SG * WPS
                    # stage subgroup rows to partition base 0 (transpose-mode
                    # matmuls only work at row base 0 on HW)
                    ecopy = epool.tile([8, 4 * LNM], BF16, tag="ecopy")
                    for c in range(4):
                        nc.sync.dma_start(
                            ecopy[0:8, c * LNM:(c + 1) * LNM],
                            E_sb[32 * c:32 * c + 8, :])
                    etile = eps_pool.tile([D, NWSG], BF16, tag="et")
                    for c in range(4):
                        for k in range(WPS):
                            nc.tensor.transpose(
                                etile[:, (c * WPS + k) * 8:
                                      (c * WPS + k) * 8 + 8],
                                ecopy[0:8, c * LNM + 128 * k:
                                      c * LNM + 128 * k + 128],
                                ident8_sb[0:8, :],
                            )
                    rhs_t = rpool.tile([D, 2 * NWSG], FP8, tag="rhs")
                    nc.vector.memset(rhs_t[:, 0:NWSG], 1.0)
                    nc.vector.tensor_copy(rhs_t[:, NWSG:2 * NWSG], etile[:])

                    # --- pools matmuls ------------------------------------
                    # window (j=c*8+r, k): lhsT at xs col block j*WPS+k,
                    # e col idx = c*40 + k*8 + r; rhs = (ones[idx], e[idx]).
                    for j in range(SG):
                        c, r = j // 8, j % 8
                        s2 = (sg * SG + j) * 2
                        for k in range(WPS):
                            wi = j * WPS + k
                            idx = c * 8 * WPS + k * 8 + r
                            nc.tensor.matmul(
                                pools_bank[:, s2:s2 + 2],
                                xs_t[:, wi * D:(wi + 1) * D],
                                rhs_t[:, idx:idx + NWSG + 1:NWSG],
                                start=(k == 0), stop=(k == WPS - 1),
                            )

                    # --- max ----------------------------------------------
                    for j in range(SG):
                        g = sg * SG + j
                        b, col = g // 128, g % 128
                        scr = spool.tile([D, LFIX], BF16, tag="scr")
                        nc.vector.tensor_scalar(
                            scr[:], xt[:, j * LFIX:(j + 1) * LFIX], 1.0,
                            None, ALU.mult, ALU.max,
                            accum_out=maxT[b][:, col:col + 1],
                        )

            # ---- final stage --------------------------------------------
            with (
                tc.tile_pool(name="fin", bufs=1) as fpool,
                tc.tile_pool(name="fps", bufs=1, space="PSUM") as fps,
            ):
                for b in range(NBLK):
                    sumT = fpool.tile([128, 128], F32, name=f"sumT{b}")
                    nc.vector.tensor_copy(
                        sumT[:], pools_bank[:, b * 256:b * 256 + 256:2])
                    exT = fpool.tile([128, 128], F32, name=f"exT{b}")
                    nc.vector.tensor_copy(
                        exT[:], pools_bank[:, b * 256 + 1:b * 256 + 256:2])

                    rc = fpool.tile([128, 1], F32, name=f"rc{b}")
                    nc.vector.reciprocal(rc[:], counts_bl[b][:])
                    zt = fpool.tile([128, 1], F32, name=f"zt{b}")
                    nc.vector.tensor_sub(zt[:], zcolB[b][:], zcorr_bl[b][:])
                    rz = fpool.tile([128, 1], F32, name=f"rz{b}")
                    nc.vector.reciprocal(rz[:], zt[:])

                    exbf = fpool.tile([128, 128], BF16, name=f"exbf{b}")
                    nc.vector.tensor_copy(exbf[:], exT[:])
                    apT_ps = fps.tile([128, 128], F32, tag="apT")
                    nc.tensor.matmul(apT_ps[:], outW_sb[:], exbf[:],
                                     start=True, stop=True)
                    apT = fpool.tile([128, 128], F32, name=f"apT{b}")
                    nc.scalar.activation(apT[:], apT_ps[:], ACTF.Identity,
                                         bias=outb_sb[:], scale=1.0)

                    mean_gd = fpool.tile([128, 128], F32, name=f"mean_gd{b}")
                    tp = fps.tile([128, 128], F32, tag="tp")
                    nc.tensor.transpose(tp[:], sumT[:], ident_sb[:])
                    nc.scalar.activation(mean_gd[:], tp[:], ACTF.Identity,
                                         bias=0.0, scale=rc[:])
                    max_gd = fpool.tile([128, 128], F32, name=f"max_gd{b}")
                    tp2 = fps.tile([128, 128], F32, tag="tp")
                    nc.tensor.transpose(tp2[:], maxT[b][:], ident_sb[:])
                    nc.scalar.activation(max_gd[:], tp2[:], ACTF.Identity,
                                         bias=0.0, scale=1.0)
                    attn_gd = fpool.tile([128, 128], F32, name=f"attn_gd{b}")
                    tp3 = fps.tile([128, 128], F32, tag="tp")
                    nc.tensor.transpose(tp3[:], apT[:], ident_sb[:])
                    nc.scalar.activation(attn_gd[:], tp3[:], ACTF.Identity,
                                         bias=0.0, scale=rz[:])

                    poolsT_bf = []
                    for nm, gd in (("m", mean_gd), ("x", max_gd),
                                   ("a", attn_gd)):
                        tpp = fps.tile([128, 128], F32, tag="tp")
                        nc.tensor.transpose(tpp[:], gd[:], ident_sb[:])
                        tbf = fpool.tile([128, 128], BF16, name=f"p{nm}T{b}")
                        nc.scalar.activation(tbf[:], tpp[:], ACTF.Identity,
                                             bias=0.0, scale=1.0)
                        poolsT_bf.append(tbf)

                    hid_ps = fps.tile([128, 128], F32, tag="hid")
                    for k in range(3):
                        nc.tensor.matmul(
                            hid_ps[:], selW1_sb[:, k * D:(k + 1) * D],
                            poolsT_bf[k][:],
                            start=(k == 0), stop=(k == 2),
                        )
                    hid_bf = fpool.tile([128, 128], BF16, name=f"hid{b}")
                    nc.scalar.activation(hid_bf[:], hid_ps[:], ACTF.Relu,
                                         bias=selb1_sb[:], scale=1.0)

                    lg_ps = fps.tile([3, 128], F32, tag="lg")
                    nc.tensor.matmul(lg_ps[:], selW2_sb[:], hid_bf[:],
                                     start=True, stop=True)
                    lgT = fpool.tile([3, 128], F32, name=f"lgT{b}")
                    nc.scalar.activation(lgT[:], lg_ps[:], ACTF.Identity,
                                         bias=selb2_sb[:], scale=1.0)

                    lg_ps2 = fps.tile([128, 3], F32, tag="lgt")
                    nc.tensor.transpose(lg_ps2[:], lgT[:], ident_sb[0:3, 0:3])
                    lg = fpool.tile([128, 3], F32, name=f"lg{b}")
                    nc.scalar.activation(lg[:], lg_ps2[:], ACTF.Identity,
                                         bias=0.0, scale=1.0)

                    m3 = fpool.tile([128, 1], F32, name=f"m3{b}")
                    nc.vector.tensor_reduce(m3[:], lg[:], mybir.AxisListType.X,
                                            ALU.max)
                    nm3 = fpool.tile([128, 1], F32, name=f"nm3{b}")
                    nc.vector.tensor_scalar(nm3[:], m3[:], -1.0, None,
                                            ALU.mult)
                    ew = fpool.tile([128, 3], F32, name=f"ew{b}")
                    den = fpool.tile([128, 1], F32, name=f"den{b}")
                    nc.scalar.activation(ew[:], lg[:], ACTF.Exp,
                                         bias=nm3[:], scale=1.0,
                                         accum_out=den[:])
                    rden = fpool.tile([128, 1], F32, name=f"rden{b}")
                    nc.vector.reciprocal(rden[:], den[:])
                    w3 = fpool.tile([128, 3], F32, name=f"w3{b}")
                    nc.vector.tensor_scalar(w3[:], ew[:], rden[:], None,
                                            ALU.mult)

                    t1 = fpool.tile([128, 128], F32, name=f"t1{b}")
                    nc.vector.tensor_scalar(t1[:], mean_gd[:], w3[:, 0:1],
                                            None, ALU.mult)
                    t2 = fpool.tile([128, 128], F32, name=f"t2{b}")
                    nc.vector.scalar_tensor_tensor(
                        out=t2[:], in0=max_gd[:], scalar=w3[:, 1:2], in1=t1[:],
                        op0=ALU.mult, op1=ALU.add,
                    )
                    out_sb = fpool.tile([128, D], F32, name=f"out_sb{b}")
                    nc.vector.scalar_tensor_tensor(
                        out=out_sb[:], in0=attn_gd[:],
                        scalar=w3[:, 2:3], in1=t2[:],
                        op0=ALU.mult, op1=ALU.add,
                    )
                    nc.sync.dma_start(
                        outP[b * 128:(b + 1) * 128, :], out_sb[:]
                    )

    nc.compile()
    return nc


# --------------------------------------------------------------------------
# host orchestration
# --------------------------------------------------------------------------

_CACHE = {}


def _shard_meta(batch, G):
    starts = np.searchsorted(batch, np.arange(G + 1))
    counts = np.diff(starts).astype(np.int64)
    return starts, counts


def _prep_core_inputs(x, batch, weights, core, starts, counts, LFIX=LFIX0):
    LNM, WPS, NPADF, NW = _derived(LFIX)
    g0 = core * GPC
    cst = starts[g0:g0 + GPC + 1]
    ccounts = counts[g0:g0 + GPC]
    n_core = cst[-1] - cst[0]
    assert ccounts.max() <= LFIX, ccounts.max()

    xs_nodes = x[cst[0]:cst[-1]]
    local_g = np.repeat(np.arange(GPC), ccounts)
    intra = np.arange(n_core) - np.repeat(cst[:-1] - cst[0], ccounts)

    xbf = xs_nodes.astype(ml_dtypes.bfloat16)

    # fm copy
    destf = local_g * LFIX + intra
    xpf = np.zeros((GPC * LFIX, D), dtype=ml_dtypes.bfloat16)
    xpf[destf] = xbf
    xTa = np.ascontiguousarray(xpf.T)

    # nm staged copy (640-pad)
    destn = local_g * LNM + intra
    xpn = np.zeros((GPC * LNM, D), dtype=FP8NP)
    xpn[destn] = xs_nodes.astype(FP8NP)
    xsa = np.ascontiguousarray(
        xpn.reshape(NW, 128, D).transpose(1, 0, 2).reshape(128, NW * D))

    (att_W, att_b, att_ctx, out_W, out_b,
     sel_W1, sel_b1, sel_W2, sel_b2) = weights

    ctxoh = np.zeros((D, 16 * 32), np.float32)
    for half in range(2):
        for r in range(8):
            v = half * 8 + r
            ctxoh[half * 64:(half + 1) * 64, v * 32 + r] = att_ctx

    s_pad = float(np.tanh(att_b.astype(np.float64)) @
                  att_ctx.astype(np.float64))
    e_pad = math.exp(s_pad)
    zcorr = ((LFIX - ccounts) * e_pad).astype(np.float32).reshape(GPC, 1)

    return {
        "xT": xTa,
        "xs": xsa,
        "attW": att_W.astype(ml_dtypes.bfloat16),
        "attb2": np.concatenate([att_b, att_b]).astype(np.float32)
                  .reshape(D, 1),
        "ctxoh": ctxoh.astype(ml_dtypes.bfloat16),
        "ident8": np.tile(np.eye(8, dtype=np.float32), (16, 1))
                   .astype(ml_dtypes.bfloat16),
        "outW": out_W.astype(ml_dtypes.bfloat16),
        "outb": out_b.astype(np.float32).reshape(D, 1),
        "selW1": sel_W1.astype(ml_dtypes.bfloat16),
        "selb1": sel_b1.astype(np.float32).reshape(D, 1),
        "selW2": sel_W2.astype(ml_dtypes.bfloat16),
        "selb2": sel_b2.astype(np.float32).reshape(3, 1),
        "counts": ccounts.astype(np.float32).reshape(GPC, 1),
        "zcorr": zcorr,
        "ident": np.eye(D, dtype=np.float32),
    }


def _run(x, batch, att_W, att_b, att_ctx, out_W, out_b,
         sel_W1, sel_b1, sel_W2, sel_b2, num_graphs, **spmd_kwargs):
    x = np.asarray(x)
    batch = np.asarray(batch).astype(np.int64)
    G = int(num_graphs)
    assert G == 2048 and x.shape == (1000000, D)

    starts, counts = _shard_meta(batch, G)

    LFIX = LFIX0
    cmax = int(counts.max())
    while cmax > LFIX:
        LFIX += 64
    key = ("nc", LFIX)
    if key not in _CACHE:
        _CACHE[key] = build_nc_v2(LFIX=LFIX)
    nc = _CACHE[key]
    _CACHE["nc"] = nc

    weights = (np.asarray(att_W), np.asarray(att_b), np.asarray(att_ctx),
               np.asarray(out_W), np.asarray(out_b),
               np.asarray(sel_W1), np.asarray(sel_b1),
               np.asarray(sel_W2), np.asarray(sel_b2))

    in_maps = [
        _prep_core_inputs(x, batch, weights, c, starts, counts, LFIX=LFIX)
        for c in range(NCORES)
    ]

    res = run_bass_kernel_spmd(nc, in_maps, core_ids=list(range(NCORES)),
                               **spmd_kwargs)
    outs = [np.asarray(res.results[c]["out"], dtype=np.float32)
            for c in range(NCORES)]
    return np.concatenate(outs, axis=0), res


def kernel(**inputs):
    return _run(**inputs)[0]
